# revision 1
# baseline (speedup 1.0000x reference)
"""Trainium2 Bass kernel for the IsLandLoss nn.Module (center loss + island loss).

Math (matches the jax reference):
  center_loss = sum((feat - centers[label])**2) / 2 / B
  island_loss = sum_{j != k} (cos(c_j, c_k) + 1)
              = ||sum_j chat_j||^2 - sum_j ||chat_j||^2 + (N^2 - N)
    where chat_j = c_j / max(||c_j||, eps)
  out = center_loss + 0.5 * island_loss

The ||.||^2-of-sum identity removes the [1000,1000] Gram matmul entirely.

Sharding: feat/label split along batch over 8 cores (4096 rows each);
centers replicated as a bf16 table padded to 1024 rows (zero rows
normalize to 0 and contribute nothing; bf16 quantization of centers
perturbs the loss by ~1e-5 relative, far below fp32 tolerance, and
halves the dominant gather traffic). Each core gathers its per-sample
center rows from HBM with a SWDGE dma_gather (1KB/row), computes its
partial sum((f-c)^2) with DVE subtract + ACT square-rowsum, and
redundantly computes the tiny island term. Per-core outputs
[center_partial, island] are combined on the host (the gather step).
"""

from contextlib import ExitStack

import ml_dtypes
import numpy as np

import concourse.bacc as bacc
import concourse.bass as bass
import concourse.mybir as mybir
from concourse import library_config, tile
from concourse.bass_utils import run_bass_kernel_spmd

N_CORES = 8
BATCH = 32768
D = 512
NCLS = 1000
NPAD = 1024  # centers padded to a multiple of 128
SHARD = BATCH // N_CORES  # 4096 rows per core
LAMDA = 0.5
EPS = 1e-8
CHUNK = 256  # rows per pipeline chunk
N_CHUNKS = SHARD // CHUNK  # 16
CPC = CHUNK // 128  # 2 row-groups of 128 per chunk
CGRP = NPAD // 128  # 8 row-groups of 128 in padded centers
FP32 = mybir.dt.float32
BF16 = mybir.dt.bfloat16
IO_BUFS = 4
SCRATCH_BUFS = 3

_cached = {}


def _build(repeat=1):
    nc = bacc.Bacc(trn_type="TRN2")

    feat_in = nc.declare_dram_parameter("feat", [SHARD, D], FP32, isOutput=False)
    idx_in = nc.declare_dram_parameter(
        "idx", [128, SHARD // 16], mybir.dt.int16, isOutput=False
    )
    cb_in = nc.declare_dram_parameter("cb16", [NPAD, D], BF16, isOutput=False)
    out_dram = nc.declare_dram_parameter("out", [1, 2], FP32, isOutput=True)

    # Partition p holds feat rows p*32..p*32+31 -> contiguous 64KB per
    # partition (efficient descriptors). Host permutes the gather indices so
    # slot i=(g*128+p) carries label[p*32+g], keeping feat/center rows paired.
    fv = feat_in[:, :].rearrange("(p g) d -> p g d", p=128)
    cv = cb_in[:, :].rearrange("(p g) d -> p g d", p=128)

    ncols = repeat * N_CHUNKS + 1  # stats columns (+1 for the trace col)

    with tile.TileContext(nc) as tc, ExitStack() as ctx:
        io_pool = ctx.enter_context(tc.tile_pool(name="io", bufs=IO_BUFS))
        scratch = ctx.enter_context(tc.tile_pool(name="scratch", bufs=SCRATCH_BUFS))
        singles = ctx.enter_context(tc.tile_pool(name="singles", bufs=1))
        psum_pool = ctx.enter_context(tc.tile_pool(name="psum", bufs=1, space="PSUM"))

        # dma_gather is an extended GPSIMD instruction: needs the attnmlp ucode lib
        nc.gpsimd.load_library(library_config.attnmlp)

        # ---- constants / small persistent tiles ----
        idx_t = singles.tile([128, SHARD // 16], mybir.dt.int16)
        nc.sync.dma_start(idx_t[:, :], idx_in[:, :])
        ones = singles.tile([128, 1], FP32)
        nc.vector.memset(ones[:, :], 1.0)
        # stats cols: per-chunk center partials, then the trace col last
        stats = singles.tile([128, ncols], FP32)

        # ---- island: per-row norms of centers, s = sum_j c_j/||c_j|| ----
        ctile = singles.tile([128, CGRP, D], BF16)
        nc.sync.dma_start(ctile[:, :, :], cv[:, :, :])
        ss = singles.tile([128, CGRP], FP32)  # per-row sum of squares
        for g in range(CGRP):
            sq_c = scratch.tile([128, D], FP32, tag="sq_c")
            nc.scalar.activation(
                sq_c[:, :],
                ctile[:, g, :],
                mybir.ActivationFunctionType.Square,
                accum_out=ss[:, g : g + 1],
            )
        w = singles.tile([128, CGRP], FP32)  # 1 / max(||c||, eps)
        nc.scalar.sqrt(w[:, :], ss[:, :])
        nc.vector.tensor_scalar_max(w[:, :], w[:, :], EPS)
        nc.vector.reciprocal(w[:, :], w[:, :])
        # trace col: sum_g ss*w*w
        t_full = singles.tile([128, CGRP], FP32)
        nc.vector.tensor_mul(t_full[:, :], ss[:, :], w[:, :])
        nc.vector.tensor_mul(t_full[:, :], t_full[:, :], w[:, :])
        nc.vector.reduce_sum(
            stats[:, ncols - 1 : ncols], t_full[:, :], axis=mybir.AxisListType.X
        )
        # s[1, D] = sum_g w_g^T @ C_g  (contraction over the 128 partitions)
        w_bf = singles.tile([128, CGRP], BF16)
        nc.vector.tensor_copy(w_bf[:, :], w[:, :])
        s_psum = psum_pool.tile([128, D], FP32, tag="s")
        for g in range(CGRP):
            nc.tensor.matmul(
                s_psum[:1, :],
                w_bf[:, g : g + 1],
                ctile[:, g, :],
                start=(g == 0),
                stop=(g == CGRP - 1),
            )

        # ---- center loss main loop ----
        for r in range(repeat):
            for c in range(N_CHUNKS):
                fch = io_pool.tile([128, CPC, D], FP32, tag="feat")
                nc.sync.dma_start(fch[:, :, :], fv[:, bass.ts(c, CPC), :])
                ft = fch[:, :, :]
                gt = io_pool.tile([128, CPC, D], BF16, tag="gath")
                nc.gpsimd.dma_gather(
                    gt[:, :, :],
                    cb_in[:, :],
                    idx_t[:, bass.ts(c, CHUNK // 16)],
                    CHUNK,
                    CHUNK,
                    D,
                )
                diff = scratch.tile([128, CPC, D], FP32, tag="diff")
                nc.vector.tensor_sub(diff[:, :, :], ft, gt[:, :, :])
                # square in place; accum_out gets the per-partition row sum
                nc.scalar.activation(
                    diff[:, :, :],
                    diff[:, :, :],
                    mybir.ActivationFunctionType.Square,
                    accum_out=stats[:, r * N_CHUNKS + c : r * N_CHUNKS + c + 1],
                )

        # ---- reductions to scalars ----
        # partials[1, k] = column sums of stats over partitions
        p_psum = psum_pool.tile([128, ncols], FP32, tag="p")
        nc.tensor.matmul(p_psum[:1, :], ones[:, :], stats[:, :], start=True, stop=True)
        partials = singles.tile([1, ncols], FP32)
        nc.vector.tensor_copy(partials[:1, :], p_psum[:1, :])

        s_sq = singles.tile([1, D], FP32)
        a_sb = singles.tile([1, 1], FP32)  # ||s||^2
        nc.scalar.activation(
            s_sq[:1, :],
            s_psum[:1, :],
            mybir.ActivationFunctionType.Square,
            accum_out=a_sb[:1, :1],
        )

        out_sb = singles.tile([1, 2], FP32)
        # col 0: raw center-loss partial sum (summed over all repeats)
        nc.vector.reduce_sum(
            out_sb[:1, 0:1], partials[:1, 0 : ncols - 1], axis=mybir.AxisListType.X
        )
        # col 1: island = ||s||^2 - trace + (N^2 - N)
        isl = singles.tile([1, 1], FP32)
        nc.vector.tensor_sub(
            isl[:1, :1], a_sb[:1, :1], partials[:1, ncols - 1 : ncols]
        )
        nc.vector.tensor_scalar_add(
            out_sb[:1, 1:2], isl[:1, :1], float(NCLS * NCLS - NCLS)
        )

        nc.sync.dma_start(out_dram[:, :], out_sb[:1, :])

    nc.compile()
    return nc


def _get_nc(repeat=1):
    if repeat not in _cached:
        _cached[repeat] = _build(repeat)
    return _cached[repeat]


def _wrap_idx(label_shard: np.ndarray) -> np.ndarray:
    # Slot i=(g*128+p) must carry the label of feat row p*32+g (the
    # contiguous-per-partition feat layout), then wrap: slot i lives at
    # [i % 16, i // 16] int16, replicated 8x across partition groups.
    perm = label_shard.reshape(128, SHARD // 128).T.reshape(-1)
    wrapped = perm.astype(np.int16).reshape(SHARD // 16, 16).T
    return np.ascontiguousarray(np.tile(wrapped, (8, 1)))


def _make_in_maps(label, feat, centers):
    feat = np.ascontiguousarray(np.asarray(feat, dtype=np.float32))
    cb16 = np.zeros((NPAD, D), dtype=ml_dtypes.bfloat16)
    cb16[:NCLS] = np.asarray(centers, dtype=np.float32).astype(ml_dtypes.bfloat16)
    label = np.asarray(label)
    return [
        {
            "feat": feat[k * SHARD : (k + 1) * SHARD],
            "idx": _wrap_idx(label[k * SHARD : (k + 1) * SHARD]),
            "cb16": cb16,
        }
        for k in range(N_CORES)
    ]


def kernel(label, feat, centers):
    in_maps = _make_in_maps(label, feat, centers)
    nc = _get_nc()
    results = run_bass_kernel_spmd(nc, in_maps, list(range(N_CORES))).results

    center_raw = np.float64(0.0)
    for k in range(N_CORES):
        center_raw += np.float64(results[k]["out"][0, 0])
    island = np.float64(results[0]["out"][0, 1])
    total = center_raw / 2.0 / BATCH + LAMDA * island
    return np.float32(total)

